# revision 43
# baseline (speedup 1.0000x reference)
"""Causal multi-head attention on 8 Trainium2 NeuronCores.

Problem: B=4, T=2048, C=1024, H=16 heads, D=64, fp32.
Sharding: 4-way data parallel on batch x 2-way tensor parallel on heads.
Core c -> batch c//2, heads (c%2)*8 .. (c%2)*8+7.

Per-core dataflow:
  Q/K/V projections run in fp8e4 DoubleRow mode (K=256 per pass, 0.5
  cycles/row = 4x bf16 throughput per pass). Accuracy is preserved with an
  error-compensated hi/lo split done on the host at scale h=16:
    Xh = fp8(16 x), Xl = fp8(16 x - float(Xh))     (same for each W)
    x@W ~= [Xh@Wh + Xl@Wh + Xh@Wl] / 256
  The three terms share a PSUM accumulation (uniform scale 256) and add
  ~0.07% relative error - below bf16 rounding. 12 DR passes replace 8
  bf16 passes: 25% fewer PE cycles on all QKV projections.

  Attention (per head pair j, 512-query block qb, 128-key tiles k):
    ST(k,q) = KT_h.T @ QT_h  (K=64; two heads on PE row groups 0-63/64-127)
    PT = exp(ST/8) on ScalarE (scores ~N(0,1): no max-subtraction)
    causal mask: DVE multiply on the single crossing 128x128 block only
    AV flipped: P is the STATIONARY operand - per 128-query tile,
      out[q, 0:65] += P_tile.T @ [V|1]  (M=128 queries fully used vs 65
      in the d-major orientation; col 64 accumulates the softmax sum l).
      PSUM accumulation groups are bank-scoped on TRN2 (a start=True wipes
      other open groups in its bank), so the four query-tile accumulators
      sharing a bank are DVE-zeroed once and every AV matmul accumulates
      with start=False.
    normalize on DVE: A[q, d] = out[q, d] * recip(l[q])  (per-partition
      broadcast - no K=1 broadcast matmuls)
    PE-transpose A (identity matmul) back to [dl, q] for the Y projection
  yT(o,t) = woT.T @ AT  -> bf16 partial output, host sums the 2 TP cores.

Scheduling: the k-loop software-pipelines score->exp->AV (AV two steps
behind the score matmuls) and streams projection / transpose / Y-proj
"fillers" into each step, paced by the exact ACT-minus-PE cycle deficit so
the filler queue lasts through the supply-poor late blocks. Transposes of
pairs 0-2 are deferred to the end of the queue for the same reason. DMA
loads are ordered per-queue around the shared HWDGE so the first
projection passes start ~3us in.
"""

import numpy as np
import ml_dtypes

B, T, C = 4, 2048, 1024
H, D = 16, 64
HL = 8           # local heads per core
DL = HL * D      # 512 local channels
N_CORES = 8
QB = 512         # query block
NQB = T // QB    # 4 query blocks
NJ = HL // 2     # 4 head pairs
NKT = 16         # fp8 stack k-tiles (8 hi + 8 lo)
BF16 = ml_dtypes.bfloat16
F8 = ml_dtypes.float8_e4m3
HSC = 16.0                 # fp8 hi scale
OSC = 1.0 / (HSC * HSC)    # projection output unscale (1/256)

# DoubleRow pass table: (w k-tile base, x k-tile base); each pass
# contracts 256 rows. Term order hi*hi, lo(w)*hi(x), hi(w)*lo(x) keeps the
# first 8 passes independent of the x-lo tiles, which arrive last at startup.
DR_PASSES = (
    [(a, a) for a in range(0, 8, 2)]
    + [(a + 8, a) for a in range(0, 8, 2)]
    + [(a, a + 8) for a in range(0, 8, 2)]
)

_CACHE: dict = {}

T_EXP = 185
T_INS = 4
T_QBS = [0, 2, 1, 3]


def _build_nc():
    import concourse.bass as bass
    from concourse import bacc, mybir, tile

    f32 = mybir.dt.float32
    bf16 = mybir.dt.bfloat16
    f8 = mybir.dt.float8e4
    EXP = mybir.ActivationFunctionType.Exp
    DR = mybir.MatmulPerfMode.DoubleRow

    nc = bacc.Bacc("TRN2", target_bir_lowering=False, debug=False)

    xs_d = nc.dram_tensor("xs", [2 * C, T], f8, kind="ExternalInput").ap()
    wqs_d = nc.dram_tensor("wqs", [2 * C, DL], f8, kind="ExternalInput").ap()
    wks_d = nc.dram_tensor("wks", [2 * C, DL], f8, kind="ExternalInput").ap()
    wvs_d = nc.dram_tensor("wvs", [2 * C, DL], f8, kind="ExternalInput").ap()
    wo_d = nc.dram_tensor("wot", [DL, C], bf16, kind="ExternalInput").ap()
    yT_d = nc.dram_tensor("yt", [C, T], bf16, kind="ExternalOutput").ap()

    def dview(d, p0, nk, c0, ncols, row_len):
        # [128, nk, ncols] view of dram [rows, row_len]: row = 128*k + p
        return bass.AP(
            tensor=d.tensor,
            offset=d.offset + p0 * 128 * row_len + c0,
            ap=[[row_len, 128], [128 * row_len, nk], [1, ncols]],
        )

    with tile.TileContext(nc) as tc:
        with (
            tc.tile_pool(name="const", bufs=1) as const,
            tc.tile_pool(name="ps", bufs=2, space="PSUM") as ps_pool,
            tc.tile_pool(name="ot", bufs=2, space="PSUM") as ot_pool,
            tc.tile_pool(name="pt", bufs=8) as pt_pool,
            tc.tile_pool(name="small", bufs=4) as small,
            tc.tile_pool(name="apool", bufs=16) as apool,
            tc.tile_pool(name="ystage", bufs=4) as ystage,
        ):
            xs_sb = const.tile([128, NKT, T], f8)
            wqs_sb = const.tile([128, NKT, DL], f8)
            wks_sb = const.tile([128, NKT, DL], f8)
            wvs_sb = const.tile([128, NKT, DL], f8)
            wo_sb = const.tile([128, DL // 128, C], bf16)
            QT_sb = const.tile([128, NJ, T], bf16)
            KT_sb = const.tile([128, NJ, T], bf16)
            V_sb = const.tile([128, T // 128, HL, D + 1], bf16)
            AT_sb = const.tile([128, NJ, T], bf16)
            mask_sb = const.tile([128, 128], bf16)
            ident_sb = const.tile([128, 128], bf16)

            nc.vector.memset(V_sb[:, :, :, D : D + 1], 1.0)
            # causal mask for the crossing 128x128 block: keep f >= p
            nc.vector.memset(mask_sb[:], 1.0)
            nc.gpsimd.affine_select(
                out=mask_sb[:],
                in_=mask_sb[:],
                pattern=[[1, 128]],
                compare_op=mybir.AluOpType.is_ge,
                fill=0.0,
                base=0,
                channel_multiplier=-1,
            )
            # identity for PE transposes: keep f == p
            nc.vector.memset(ident_sb[:], 1.0)
            nc.gpsimd.affine_select(
                out=ident_sb[:],
                in_=ident_sb[:],
                pattern=[[1, 128]],
                compare_op=mybir.AluOpType.is_ge,
                fill=0.0,
                base=0,
                channel_multiplier=-1,
            )
            nc.gpsimd.affine_select(
                out=ident_sb[:],
                in_=ident_sb[:],
                pattern=[[-1, 128]],
                compare_op=mybir.AluOpType.is_ge,
                fill=0.0,
                base=0,
                channel_multiplier=1,
            )

            # ---- input loads. The shared HWDGE serializes descriptor
            # generation (~0.6us fixed + 0.34ns/desc per dma_start), so the
            # startup-critical set is a few lean DMAs in consumption order;
            # the bulk rides the software DGE (gpsimd) which only starts
            # paying off after its ~2.5us spin-up. ----
            nc.scalar.dma_start(
                wks_sb[:, 0:2, 0:128], dview(wks_d, 0, 2, 0, 128, DL)
            )
            nc.sync.dma_start(xs_sb[:, 0:2, 0:QB], dview(xs_d, 0, 2, 0, QB, T))
            nc.scalar.dma_start(
                wks_sb[:, 2:16, 0:128], dview(wks_d, 2, 14, 0, 128, DL)
            )
            nc.sync.dma_start(xs_sb[:, 2:8, 0:QB], dview(xs_d, 2, 6, 0, QB, T))
            nc.scalar.dma_start(
                wqs_sb[:, :, 0:128], dview(wqs_d, 0, NKT, 0, 128, DL)
            )
            nc.sync.dma_start(
                xs_sb[:, 8:16, 0:QB], dview(xs_d, 8, 8, 0, QB, T)
            )
            nc.scalar.dma_start(
                wvs_sb[:, 0:8, :], dview(wvs_d, 0, 8, 0, DL, DL)
            )
            nc.scalar.dma_start(
                wvs_sb[:, 8:16, :], dview(wvs_d, 8, 8, 0, DL, DL)
            )
            # software DGE: x t1/t2/t3 in hi/lo halves (completion sems fire
            # per dma_start, so halves unblock consumers earlier)
            nc.gpsimd.dma_start(
                xs_sb[:, 0:8, QB : 2 * QB], dview(xs_d, 0, 8, QB, QB, T)
            )
            nc.gpsimd.dma_start(
                xs_sb[:, 8:16, QB : 2 * QB], dview(xs_d, 8, 8, QB, QB, T)
            )
            nc.gpsimd.dma_start(
                xs_sb[:, 0:8, 2 * QB : 3 * QB], dview(xs_d, 0, 8, 2 * QB, QB, T)
            )
            nc.gpsimd.dma_start(
                xs_sb[:, 8:16, 2 * QB : 3 * QB], dview(xs_d, 8, 8, 2 * QB, QB, T)
            )
            nc.gpsimd.dma_start(
                xs_sb[:, 0:8, 3 * QB : 4 * QB], dview(xs_d, 0, 8, 3 * QB, QB, T)
            )
            nc.gpsimd.dma_start(
                xs_sb[:, 8:16, 3 * QB : 4 * QB],
                dview(xs_d, 8, 8, 3 * QB, QB, T),
            )
            nc.gpsimd.dma_start(
                wks_sb[:, :, 128:DL], dview(wks_d, 0, NKT, 128, DL - 128, DL)
            )
            nc.gpsimd.dma_start(
                wqs_sb[:, :, 128:DL], dview(wqs_d, 0, NKT, 128, DL - 128, DL)
            )
            for r in range(DL // 128):
                nc.gpsimd.dma_start(
                    wo_sb[:, r, :], wo_d[r * 128 : (r + 1) * 128, :]
                )

            def proj_qk_block(w_sb, out_sb, j, tb):
                # (dl, t) projection for head pair j, one 512-col t block
                acc = ps_pool.tile([128, QB], f32, tag="ps")
                dls = slice(j * 128, (j + 1) * 128)
                ts = slice(tb * QB, (tb + 1) * QB)
                for i, (a, b) in enumerate(DR_PASSES):
                    nc.tensor.matmul(
                        acc[:],
                        lhsT=w_sb[:, a : a + 2, dls],
                        rhs=xs_sb[:, b : b + 2, ts],
                        start=(i == 0),
                        stop=(i == len(DR_PASSES) - 1),
                        perf_mode=DR,
                    )
                    yield 256
                nc.vector.tensor_scalar_mul(out_sb[:, j, ts], acc[:], OSC)

            def proj_v_block(tt):
                # V natural: (t, dl) for one 128-row t tile, all heads
                acc = ps_pool.tile([128, DL], f32, tag="ps")
                ts = slice(tt * 128, (tt + 1) * 128)
                for i, (a, b) in enumerate(DR_PASSES):
                    nc.tensor.matmul(
                        acc[:],
                        lhsT=xs_sb[:, b : b + 2, ts],
                        rhs=wvs_sb[:, a : a + 2, :],
                        start=(i == 0),
                        stop=(i == len(DR_PASSES) - 1),
                        perf_mode=DR,
                    )
                    yield 256
                nc.vector.tensor_scalar_mul(
                    V_sb[:, tt, :, 0:D],
                    acc.rearrange("p (h d) -> p h d", h=HL),
                    OSC,
                )

            def proj_y_block(qb, ob):
                q0 = qb * QB
                acc = ps_pool.tile([128, QB], f32, tag="ps")
                for r in range(DL // 128):
                    nc.tensor.matmul(
                        acc[:],
                        lhsT=wo_sb[:, r, ob * 128 : (ob + 1) * 128],
                        rhs=AT_sb[:, r, q0 : q0 + QB],
                        start=(r == 0),
                        stop=(r == DL // 128 - 1),
                    )
                    yield 512
                yst = ystage.tile([128, QB], bf16, tag="yst")
                nc.vector.tensor_copy(yst[:], acc[:])
                nc.sync.dma_start(
                    yT_d[ob * 128 : (ob + 1) * 128, q0 : q0 + QB], yst[:]
                )

            def tp_block(j, qb, a_sb):
                # transpose A [q, dl] -> AT [dl, q] for head pair j
                q0 = qb * QB
                for qt in range(4):
                    tp = ps_pool.tile([128, 128], bf16, tag="ps", name="tp")
                    nc.tensor.transpose(tp[:], a_sb[:, qt, :], ident_sb[:])
                    yield 128
                    nc.vector.tensor_copy(
                        AT_sb[:, j, q0 + qt * 128 : q0 + (qt + 1) * 128], tp[:]
                    )

            # ---- filler machinery: a queue of generators streamed into the
            # attention k-loop as PE gap filler ----
            filler: dict = {"items": [], "idx": 0, "done": set()}

            def filler_add(name, gen):
                filler["items"].append((name, gen))

            def filler_insert(name, gen):
                # insert a few generators behind the in-flight one: soon, but
                # late enough that the DVE norm feeding it has drained, and
                # never interleaving with an open PSUM accumulation
                pos = min(filler["idx"] + T_INS, len(filler["items"]))
                filler["items"].insert(pos, (name, gen))

            def filler_pull(cycles):
                # pull filler work until ~`cycles` PE cycles were emitted
                pulled = 0
                while pulled < cycles and filler["idx"] < len(filler["items"]):
                    name, gen = filler["items"][filler["idx"]]
                    try:
                        pulled += next(gen)
                    except StopIteration:
                        filler["done"].add(name)
                        filler["idx"] += 1
                return pulled

            def filler_flush_until(names):
                while not all(n in filler["done"] for n in names):
                    if filler["idx"] >= len(filler["items"]):
                        missing = [n for n in names if n not in filler["done"]]
                        raise RuntimeError(f"filler queue exhausted: {missing}")
                    filler_pull(1)

            pending_norm: list = []

            def flush_norm():
                while pending_norm:
                    pending_norm.pop(0)()

            def attention(j, qb):
                q0 = qb * QB
                kb = (qb + 1) * (QB // 128)
                h0, h1 = 2 * j, 2 * j + 1
                ot0 = ot_pool.tile([128, 4, 128], f32, tag="ot", name="ot0")
                ot1 = ot_pool.tile([128, 4, 128], f32, tag="ot", name="ot1")

                def emit_st(k):
                    m = k - 4 * qb  # >=0: diagonal tile index
                    w0 = max(0, m) * 128
                    st = ps_pool.tile([128, 2, QB], f32, tag="st")
                    for hi, base in ((0, 0), (1, 64)):
                        nc.tensor.matmul(
                            st[:, hi, w0:QB],
                            lhsT=KT_sb[base : base + 64, j, k * 128 : (k + 1) * 128],
                            rhs=QT_sb[base : base + 64, j, q0 + w0 : q0 + QB],
                            start=True,
                            stop=True,
                        )
                    pt = pt_pool.tile([128, 2, QB], bf16, tag="pt")
                    # P = exp(S / sqrt(D)); scores are ~N(0,1): no
                    # max-subtraction needed
                    nc.scalar.activation(
                        pt[:, :, w0:QB], st[:, :, w0:QB], EXP, scale=0.125
                    )
                    if m >= 0:
                        # zero key > query inside the crossing 128x128 block
                        m_ap = bass.AP(
                            tensor=mask_sb.tensor,
                            offset=mask_sb.offset,
                            ap=[mask_sb.ap[0], [0, 2], [1, 128]],
                        )
                        nc.vector.tensor_mul(
                            pt[:, :, w0 : w0 + 128], pt[:, :, w0 : w0 + 128], m_ap
                        )
                    return pt, m

                def emit_av(k, pt, m):
                    if j == 0:
                        filler_flush_until([f"v{k}"])
                    # PSUM accumulation groups are bank-scoped on TRN2: a
                    # start=True wipes other open groups in the same bank.
                    # The four query-tile accumulators share a bank, so the
                    # bank is pre-zeroed (gpsimd memset) and every matmul
                    # accumulates (start=False).
                    for ot, hi, hh in ((ot0, 0, h0), (ot1, 1, h1)):
                        for qt in range(max(0, m), 4):
                            nc.tensor.matmul(
                                ot[:, qt, 0 : D + 1],
                                lhsT=pt[:, hi, qt * 128 : (qt + 1) * 128],
                                rhs=V_sb[:, k, hh, :],
                                start=False,
                                stop=(k == 4 * qb + qt),
                                skip_group_check=True,
                            )

                # norm(B-1) is pure DVE work: flushing it first lets it (and
                # the accumulator zeroing) overlap the first two score
                # matmuls; AV runs two k-steps behind the score pipeline so
                # it never waits on exp. Filler pulls track the exact
                # ACT-minus-PE deficit so supply lasts through the late
                # blocks (ACT: 0.833 ns/elem + 185 ns/instr; PE: 1 cyc/col
                # at 2.4 GHz).
                def exp_cycles(w0):
                    return int((2 * (QB - w0) * 0.8333 + T_EXP) * 2.4)

                def w0_of(k):
                    return max(0, k - 4 * qb) * 128

                def av_cycles(m):
                    return (4 - max(0, m)) * 2 * 65

                flush_norm()
                nc.vector.memset(ot0[:, :, 0 : D + 1], 0.0)
                nc.vector.memset(ot1[:, :, 0 : D + 1], 0.0)
                p0 = emit_st(0)
                p1 = emit_st(1)
                bal = sum(
                    exp_cycles(w0_of(k)) - 2 * (QB - w0_of(k)) for k in (0, 1)
                )
                bal -= filler_pull(bal)
                for k in range(2, kb):
                    cur = emit_st(k)
                    bal += exp_cycles(w0_of(k)) - 2 * (QB - w0_of(k))
                    bal -= filler_pull(bal)
                    emit_av(k - 2, *p0)
                    bal -= av_cycles(p0[1])
                    p0, p1 = p1, cur
                bal -= filler_pull(bal)
                emit_av(kb - 2, *p0)
                bal -= av_cycles(p0[1])
                bal -= filler_pull(bal)
                emit_av(kb - 1, *p1)

                def norm(ot0=ot0, ot1=ot1, j=j, qb=qb):
                    lsum = small.tile([128, 2, 4, 1], f32, tag="ls", name="lsum")
                    nc.vector.tensor_copy(lsum[:, 0], ot0[:, :, D : D + 1])
                    nc.vector.tensor_copy(lsum[:, 1], ot1[:, :, D : D + 1])
                    lrec = small.tile([128, 2, 4, 1], f32, tag="lr", name="lrec")
                    nc.vector.reciprocal(lrec[:], lsum[:])
                    a_sb = apool.tile([128, 4, 128], bf16, tag="a", name="a_sb")
                    for hi, ot in ((0, ot0), (1, ot1)):
                        lbase = lrec[:, hi]
                        lr_ap = bass.AP(
                            tensor=lbase.tensor,
                            offset=lbase.offset,
                            ap=[lbase.ap[0], lbase.ap[1], [0, D]],
                        )
                        nc.vector.tensor_mul(
                            a_sb[:, :, hi * D : (hi + 1) * D],
                            ot[:, :, 0:D],
                            lr_ap,
                        )
                    if j == NJ - 1:
                        filler_insert(f"tp{j}.{qb}", tp_block(j, qb, a_sb))
                        for ob in range(C // 128):
                            filler_add(f"y{qb}.{ob}", proj_y_block(qb, ob))
                    else:
                        # deferred: j<3 transposes are only needed before
                        # y(qb); appending sends them to the supply-starved
                        # late blocks
                        filler_add(f"tp{j}.{qb}", tp_block(j, qb, a_sb))

                pending_norm.append(norm)

            def run(gen):
                for _ in gen:
                    pass

            # Build the filler queue: everything except the j=0/qb=0
            # prerequisites, in rough just-in-time order.
            for tt in range(4):
                filler_add(f"v{tt}", proj_v_block(tt))
            for qb in range(1, NQB):
                filler_add(f"kq0.{qb}k", proj_qk_block(wks_sb, KT_sb, 0, qb))
                filler_add(f"kq0.{qb}q", proj_qk_block(wqs_sb, QT_sb, 0, qb))
                for tt in range(4 * qb, 4 * qb + 4):
                    filler_add(f"v{tt}", proj_v_block(tt))
            for j in range(1, NJ):
                for qb in range(NQB):
                    filler_add(f"kq{j}.{qb}k", proj_qk_block(wks_sb, KT_sb, j, qb))
                    filler_add(f"kq{j}.{qb}q", proj_qk_block(wqs_sb, QT_sb, j, qb))

            def need_attention(j, qb):
                if j == 0:
                    if qb == 0:
                        return []
                    names = [f"kq0.{t}k" for t in range(1, qb + 1)]
                    names += [f"kq0.{qb}q"]
                    return names
                names = [f"kq{j}.{t}k" for t in range(qb + 1)]
                names += [f"kq{j}.{qb}q"]
                return names

            # j=0/qb=0 projection prerequisites emitted directly; the V
            # blocks stream into attention(0,*) as fillers instead (each AV
            # step flushes its own V tile just-in-time)
            run(proj_qk_block(wks_sb, KT_sb, 0, 0))
            run(proj_qk_block(wqs_sb, QT_sb, 0, 0))

            for j in range(NJ):
                # last head pair runs qb order 0,2,3,1: each block's y
                # fillers unlock at the NEXT block's flush, so this order
                # keeps y supply flowing and leaves y(1) surplus to cover
                # the drain's norm->transpose latency
                qbs = T_QBS if j == NJ - 1 else range(NQB)
                for qb in qbs:
                    filler_flush_until(need_attention(j, qb))
                    attention(j, qb)
            # drain the last norm and remaining fillers (tail transposes +
            # y projections)
            flush_norm()
            filler_pull(1_000_000_000)

    nc.compile()
    return nc


def _get_nc():
    if "nc" not in _CACHE:
        _CACHE["nc"] = _build_nc()
    return _CACHE["nc"]


def _run(in_maps, trace=False):
    from concourse.bass_utils import run_bass_kernel_spmd

    nc = _get_nc()
    return run_bass_kernel_spmd(nc, in_maps, list(range(N_CORES)), trace=trace)


def _split8(a):
    # error-compensated fp8 split at scale HSC: a ~= (hi + lo) / HSC
    hi = (HSC * a).astype(F8)
    lo = (HSC * a - hi.astype(np.float32)).astype(F8)
    return np.concatenate([hi, lo], axis=0)


def _make_in_maps(x, W_Q, W_K, W_V, W_out):
    x = np.asarray(x, dtype=np.float32)
    W_Q = np.asarray(W_Q, dtype=np.float32)
    W_K = np.asarray(W_K, dtype=np.float32)
    W_V = np.asarray(W_V, dtype=np.float32)
    W_out = np.asarray(W_out, dtype=np.float32)

    xs_by_b = [_split8(np.ascontiguousarray(x[b].T)) for b in range(B)]
    in_maps = []
    for core in range(N_CORES):
        b, hh = core // 2, core % 2
        sl = slice(hh * DL, (hh + 1) * DL)
        in_maps.append(
            {
                "xs": xs_by_b[b],
                "wqs": _split8(np.ascontiguousarray(W_Q[sl, :].T)),
                "wks": _split8(np.ascontiguousarray(W_K[sl, :].T)),
                "wvs": _split8(np.ascontiguousarray(W_V[sl, :].T)),
                "wot": np.ascontiguousarray(W_out[:, sl].T).astype(BF16),
            }
        )
    return in_maps


def _assemble(results):
    y = np.empty((B, T, C), dtype=np.float32)
    for b in range(B):
        yT = results[2 * b]["yt"].astype(np.float32) + results[
            2 * b + 1
        ]["yt"].astype(np.float32)
        y[b] = yT.T
    return y


def kernel(x, W_Q, W_K, W_V, W_out):
    res = _run(_make_in_maps(x, W_Q, W_K, W_V, W_out), trace=False)
    return _assemble(res.results)
